# revision 1
# baseline (speedup 1.0000x reference)
"""Chamfer distance (CDLoss) Trainium2 kernel.

Problem: prediction [4, 8192, 3], ground_truth [4, 8192, 3] (fp32).
For each batch: d2[n,m] = max(||p_n||^2 + ||g_m||^2 - 2 p.g, 0);
out[b] = sum_n min_m d2 / N + sum_m min_n d2 / M.

Strategy (8 NeuronCores): core c handles (batch = c//2, row-half = c%2),
i.e. a 4096 x 8192 slab of the distance matrix.

Device kernel per core (32 row blocks x 16 column tiles of [128, 512]):
  - Augmented-coordinate trick: ap[5, 4096] = [px, py, pz, ||p||^2, 1],
    ag[5, 8192] = [-2gx, -2gy, -2gz, 1, ||g||^2] so a single K=5 fp32
    matmul emits a [128, 512] tile of squared distances into one PSUM
    bank (PE time ~N cycles regardless of K).
  - VectorE tensor_reduce(min) per tile: exact fp32 row-min partial per
    (rowblock, coltile) into rowparts[128, 32*16].
  - ScalarE copy: PSUM -> SBUF cast to bf16 (the only other PSUM exit).
  - VectorE tensor_tensor(min) in bf16 (2x perf mode): running
    column-min buffer [128, 8192]. bf16 min is exact-monotone
    (min of rounded = rounded min), and the final sum of 8192 values
    concentrates the rounding to ~1e-5 relative.
Host: final tiny reductions (min over 128 partitions / 16 col tiles,
relu clamp, sums) in numpy. min-then-clamp == clamp-then-min, so the
relu of the reference moves to the host gather.
"""

import numpy as np

_B = 4
_N = 8192  # points per cloud
_HALF = _N // 2  # rows per core
_RB = _HALF // 128  # 32 row blocks
_GW = 512  # column group width (one PSUM bank)
_G = _N // _GW  # 16 column groups
_NCORES = 8

_CACHED_NC = None
_RUNNERS = {}


def _build_nc(repeat=1, variant="v1", gw=None, sbufs=6):
    import concourse.bacc as bacc
    import concourse.tile as tile
    from concourse import mybir

    f32 = mybir.dt.float32
    bf16 = mybir.dt.bfloat16

    gw = gw or _GW
    n_g = _N // gw
    n_mm = gw // 512  # matmuls (N<=512 fp32) per column group

    nc = bacc.Bacc("TRN2", target_bir_lowering=False, debug=False)

    ap_d = nc.dram_tensor("ap", [5, _HALF], f32, kind="ExternalInput")
    ag_d = nc.dram_tensor("ag", [5, _N], f32, kind="ExternalInput")
    rowparts_d = nc.dram_tensor(
        "rowparts", [128, _RB * n_g], f32, kind="ExternalOutput"
    )
    colmin_d = nc.dram_tensor("colmin", [128, _N], bf16, kind="ExternalOutput")

    BIG = 1.0e38

    with tile.TileContext(nc) as tc:
        with (
            tc.tile_pool(name="singles", bufs=1) as singles,
            tc.tile_pool(name="spool", bufs=sbufs) as spool,
            tc.tile_pool(name="rpool", bufs=3) as rpool,
            tc.tile_pool(name="psum", bufs=8 // n_mm, space="PSUM") as pp,
        ):
            ap_s = singles.tile([5, _HALF], f32)
            nc.sync.dma_start(out=ap_s[:], in_=ap_d[:])
            ag_s = singles.tile([5, _N], f32)
            nc.sync.dma_start(out=ag_s[:], in_=ag_d[:])

            colmin_s = singles.tile([128, _N], bf16)
            nc.vector.memset(colmin_s[:], BIG)
            rowparts_s = singles.tile([128, _RB * n_g], f32)
            if variant == "v3":
                # v3 only writes one rowparts column per row block; fill
                # the rest with BIG so the host-side min ignores them.
                nc.vector.memset(rowparts_s[:], BIG)

            def _body():
                for rb in range(_RB):
                    lhsT = ap_s[:, rb * 128 : (rb + 1) * 128]
                    if variant == "v3":
                        rowbuf = rpool.tile([128, gw], bf16, tag="rowbuf")
                        nc.vector.memset(rowbuf[:], BIG)
                    for g in range(n_g):
                        t = pp.tile([128, gw], f32, tag="t")
                        for j in range(n_mm):
                            c0 = g * gw + j * 512
                            nc.tensor.matmul(
                                t[:, j * 512 : (j + 1) * 512],
                                lhsT,
                                ag_s[:, c0 : c0 + 512],
                                start=True,
                                stop=True,
                            )
                        idx = rb * n_g + g
                        if variant == "v1":
                            # exact fp32 row minima (DVE, PSUM src, 1x)
                            nc.vector.tensor_reduce(
                                rowparts_s[:, idx : idx + 1],
                                t[:],
                                axis=mybir.AxisListType.X,
                                op=mybir.AluOpType.min,
                            )
                        # PSUM -> SBUF exit on ScalarE, cast to bf16
                        s = spool.tile([128, gw], bf16, tag="s")
                        nc.scalar.copy(s[:], t[:])
                        if variant == "v4":
                            # row minima from the bf16 copy (SBUF src is
                            # cheaper for DVE than fp32 PSUM)
                            nc.vector.tensor_reduce(
                                rowparts_s[:, idx : idx + 1],
                                s[:],
                                axis=mybir.AxisListType.X,
                                op=mybir.AluOpType.min,
                            )
                        if variant == "v3":
                            # row minima via cheap bf16 2x tensor_tensor
                            nc.vector.tensor_tensor(
                                rowbuf[:], rowbuf[:], s[:],
                                op=mybir.AluOpType.min,
                            )
                        # running column minima (bf16, DVE 2x mode)
                        cslice = colmin_s[:, g * gw : (g + 1) * gw]
                        nc.vector.tensor_tensor(
                            cslice, cslice, s[:], op=mybir.AluOpType.min
                        )
                    if variant == "v3":
                        nc.vector.tensor_reduce(
                            rowparts_s[:, rb * n_g : rb * n_g + 1],
                            rowbuf[:],
                            axis=mybir.AxisListType.X,
                            op=mybir.AluOpType.min,
                        )

            if repeat == 1:
                _body()
            else:
                # benchmark mode: body is idempotent (mins), repeat on-device
                with tc.For_i(0, repeat, 1):
                    _body()

            nc.sync.dma_start(out=rowparts_d[:], in_=rowparts_s[:])
            nc.sync.dma_start(out=colmin_d[:], in_=colmin_s[:])

    nc.compile()
    return nc


def _get_nc():
    global _CACHED_NC
    if _CACHED_NC is None:
        _CACHED_NC = _build_nc()
    return _CACHED_NC


def _prep_core_inputs(prediction, ground_truth):
    """Build per-core augmented matrices (host-side, fp32)."""
    in_maps = []
    for c in range(_NCORES):
        b, h = divmod(c, 2)
        p = np.asarray(prediction[b, h * _HALF : (h + 1) * _HALF], dtype=np.float32)
        g = np.asarray(ground_truth[b], dtype=np.float32)
        ap = np.empty((5, _HALF), dtype=np.float32)
        ap[0:3] = p.T
        ap[3] = (p * p).sum(axis=1, dtype=np.float32)
        ap[4] = 1.0
        ag = np.empty((5, _N), dtype=np.float32)
        ag[0:3] = (-2.0 * g).T
        ag[3] = 1.0
        ag[4] = (g * g).sum(axis=1, dtype=np.float32)
        in_maps.append({"ap": ap, "ag": ag})
    return in_maps


def _make_runner(nc, n_cores):
    """Build a cached jitted SPMD executor for `nc` (axon/PJRT path).

    Mirrors concourse.bass2jax.run_bass_via_pjrt but caches the jitted
    callable so repeat calls don't re-trace/re-compile.
    """
    import jax
    import numpy as _np
    from jax.sharding import Mesh, PartitionSpec
    from jax.experimental.shard_map import shard_map
    from concourse import mybir
    from concourse.bass2jax import (
        _bass_exec_p,
        install_neuronx_cc_hook,
        partition_id_tensor,
    )

    install_neuronx_cc_hook()

    partition_name = (
        nc.partition_id_tensor.name if nc.partition_id_tensor else None
    )
    in_names, out_names, out_avals, zero_shapes = [], [], [], []
    for alloc in nc.m.functions[0].allocations:
        if not isinstance(alloc, mybir.MemoryLocationSet):
            continue
        name = alloc.memorylocations[0].name
        if alloc.kind == "ExternalInput":
            if name == partition_name:
                continue
            in_names.append(name)
        elif alloc.kind == "ExternalOutput":
            shape = tuple(alloc.tensor_shape)
            dtype = mybir.dt.np(alloc.dtype)
            out_names.append(name)
            out_avals.append(jax.core.ShapedArray(shape, dtype))
            zero_shapes.append((shape, dtype))
    n_params = len(in_names)
    n_outs = len(out_names)
    all_names = in_names + out_names
    if partition_name is not None:
        all_names = all_names + [partition_name]
    donate = tuple(range(n_params, n_params + n_outs))

    def _body(*args):
        operands = list(args)
        if partition_name is not None:
            operands.append(partition_id_tensor())
        outs = _bass_exec_p.bind(
            *operands,
            out_avals=tuple(out_avals),
            in_names=tuple(all_names),
            out_names=tuple(out_names),
            lowering_input_output_aliases=(),
            sim_require_finite=True,
            sim_require_nnan=True,
            nc=nc,
        )
        return tuple(outs)

    devices = jax.devices()[:n_cores]
    mesh = Mesh(_np.asarray(devices), ("core",))
    sharded = jax.jit(
        shard_map(
            _body,
            mesh=mesh,
            in_specs=(PartitionSpec("core"),) * (n_params + n_outs),
            out_specs=(PartitionSpec("core"),) * n_outs,
            check_rep=False,
        ),
        donate_argnums=donate,
        keep_unused=True,
    )

    def run(in_maps):
        concat_in = [
            _np.concatenate([m[name] for m in in_maps], axis=0)
            for name in in_names
        ]
        concat_zeros = [
            _np.zeros((n_cores * s[0], *s[1:]), d) for (s, d) in zero_shapes
        ]
        out_arrs = sharded(*concat_in, *concat_zeros)
        return [
            {
                name: _np.asarray(out_arrs[i]).reshape(
                    n_cores, *out_avals[i].shape
                )[c]
                for i, name in enumerate(out_names)
            }
            for c in range(n_cores)
        ]

    return run


def _get_runner(nc, n_cores=_NCORES):
    key = id(nc)
    if key not in _RUNNERS:
        _RUNNERS[key] = _make_runner(nc, n_cores)
    return _RUNNERS[key]


def kernel(prediction, ground_truth):
    prediction = np.asarray(prediction, dtype=np.float32)
    ground_truth = np.asarray(ground_truth, dtype=np.float32)

    nc = _get_nc()
    in_maps = _prep_core_inputs(prediction, ground_truth)
    results = _get_runner(nc)(in_maps)

    out = np.zeros(_B, dtype=np.float32)
    for b in range(_B):
        dx = 0.0
        cms = []
        for h in range(2):
            r = results[2 * b + h]
            # rowparts[p, rb*G + g] = min over group g of row rb*128+p
            rp = r["rowparts"].reshape(128, _RB, _G).min(axis=2)  # [128, RB]
            dx += np.maximum(rp, 0.0).sum(dtype=np.float64)
            # colmin[p, j] = min over this core's row-blocks (partition p)
            cms.append(r["colmin"].astype(np.float32).min(axis=0))  # [N]
        cm = np.minimum(cms[0], cms[1])
        dy = np.maximum(cm, 0.0).sum(dtype=np.float64)
        out[b] = dx / _N + dy / _N
    return out



# revision 2
# speedup vs baseline: 1.4619x; 1.4619x over previous
"""Chamfer distance (CDLoss) Trainium2 kernel, v2.

Problem: prediction [4, 8192, 3], ground_truth [4, 8192, 3] (fp32).
For each batch: d2[n,m] = max(||p_n||^2 + ||g_m||^2 - 2 p.g, 0);
out[b] = sum_n min_m d2 / N + sum_m min_n d2 / M.

Strategy (8 NeuronCores): core c handles (batch = c//2, row-half = c%2),
i.e. a 4096 x 8192 slab of the distance matrix.

v2 design (all rates HW-measured via mb.py):
  - NEGATED distances: host builds ap=[px,py,pz,||p||^2,1],
    ag=[2gx,2gy,2gz,-1,-||g||^2] so one K=5 fp32 matmul emits
    -d2 tiles; every reduction is then a native MAX (enables pool_max).
  - PE: 16 matmuls [128,512] per row block into two 4-bank PSUM wide
    tiles [128,2048] (double buffered).
  - ScalarE: batched PSUM exit, one copy per wide tile f32->bf16
    (~2.6us per 2048 = 660ns/tile-equiv; FD=512 copies cost 940ns).
  - VectorE: running column-max fold, 16x tensor_tensor FD=512 bf16
    (~510ns each, 2x mode); row max via 8x pool_max w=1024 (variant
    "pool", ~700ns each at 4x) or 16x rowbuf TT (variant "ttrow")
    + one small tensor_reduce per row block.
Host: negate, clamp, fold 128 partitions / two halves, final sums.
"""

import numpy as np

_B = 4
_N = 8192  # points per cloud
_HALF = _N // 2  # rows per core
_RB = _HALF // 128  # 32 row blocks
_WIDE = 2048  # ScalarE exit width (4 PSUM banks)
_NW = _N // _WIDE  # 4 wide groups per row block
_NCORES = 8
_BIG = 1.0e30

_CACHED_NC = None
_RUNNERS = {}


def _build_nc(repeat=1, variant="pool"):
    import concourse.bacc as bacc
    import concourse.tile as tile
    from concourse import mybir

    f32 = mybir.dt.float32
    bf16 = mybir.dt.bfloat16
    MAX = mybir.AluOpType.max

    nc = bacc.Bacc("TRN2", target_bir_lowering=False, debug=False)

    ap_d = nc.dram_tensor("ap", [5, _HALF], f32, kind="ExternalInput")
    ag_d = nc.dram_tensor("ag", [5, _N], f32, kind="ExternalInput")
    rowparts_d = nc.dram_tensor("rowparts", [128, _RB], bf16, kind="ExternalOutput")
    colmax_d = nc.dram_tensor("colmax", [128, _N], bf16, kind="ExternalOutput")

    with tile.TileContext(nc) as tc:
        with (
            tc.tile_pool(name="singles", bufs=1) as singles,
            tc.tile_pool(name="slabs", bufs=2) as slabs,
            tc.tile_pool(name="rpool", bufs=2) as rpool,
            tc.tile_pool(name="psum", bufs=2, space="PSUM") as pp,
        ):
            ap_s = singles.tile([5, _HALF], f32)
            nc.sync.dma_start(out=ap_s[:], in_=ap_d[:])
            ag_s = singles.tile([5, _N], f32)
            nc.sync.dma_start(out=ag_s[:], in_=ag_d[:])

            colmax_s = singles.tile([128, _N], bf16)
            nc.vector.memset(colmax_s[:], -_BIG)
            rowparts_s = singles.tile([128, _RB], bf16)

            def _body():
                for rb in range(_RB):
                    lhsT = ap_s[:, rb * 128 : (rb + 1) * 128]
                    slab = slabs.tile([128, _N], bf16, tag="slab")
                    for w in range(_NW):
                        t = pp.tile([128, _WIDE], f32, tag="t")
                        for j in range(_WIDE // 512):
                            c0 = w * _WIDE + j * 512
                            nc.tensor.matmul(
                                t[:, j * 512 : (j + 1) * 512],
                                lhsT,
                                ag_s[:, c0 : c0 + 512],
                                start=True,
                                stop=True,
                            )
                        # batched PSUM exit on ScalarE, f32 -> bf16
                        nc.scalar.copy(slab[:, w * _WIDE : (w + 1) * _WIDE], t[:])
                        # running column-max folds (DVE, 2x bf16)
                        for j in range(_WIDE // 512):
                            c0 = w * _WIDE + j * 512
                            cs = colmax_s[:, c0 : c0 + 512]
                            nc.vector.tensor_tensor(
                                cs, cs, slab[:, c0 : c0 + 512], op=MAX
                            )
                    # row max of this 128-row block
                    if variant == "pool":
                        parts = rpool.tile([128, 8], bf16, tag="parts")
                        for q in range(8):
                            win = slab[:, q * 1024 : (q + 1) * 1024].rearrange(
                                "p (n w) -> p n w", w=1024
                            )
                            nc.vector.pool(
                                parts[:, q : q + 1], win,
                                func=mybir.PoolFunctionType.max,
                            )
                        nc.vector.tensor_reduce(
                            rowparts_s[:, rb : rb + 1], parts[:],
                            axis=mybir.AxisListType.X, op=MAX,
                        )
                    else:  # "ttrow"
                        rowbuf = rpool.tile([128, 512], bf16, tag="rowbuf")
                        nc.vector.tensor_copy(rowbuf[:], slab[:, 0:512])
                        for j in range(1, 16):
                            nc.vector.tensor_tensor(
                                rowbuf[:], rowbuf[:],
                                slab[:, j * 512 : (j + 1) * 512], op=MAX,
                            )
                        nc.vector.tensor_reduce(
                            rowparts_s[:, rb : rb + 1], rowbuf[:],
                            axis=mybir.AxisListType.X, op=MAX,
                        )

            if repeat == 1:
                _body()
            else:
                # benchmark mode: body is idempotent (maxes), repeat on-device
                with tc.For_i(0, repeat, 1):
                    _body()

            nc.sync.dma_start(out=rowparts_d[:], in_=rowparts_s[:])
            nc.sync.dma_start(out=colmax_d[:], in_=colmax_s[:])

    nc.compile()
    return nc


def _get_nc():
    global _CACHED_NC
    if _CACHED_NC is None:
        _CACHED_NC = _build_nc()
    return _CACHED_NC


def _prep_core_inputs(prediction, ground_truth):
    """Build per-core augmented matrices (host-side, fp32), NEGATED form:
    ap.T @ ag = 2 p.g - ||p||^2 - ||g||^2 = -d2."""
    in_maps = []
    for c in range(_NCORES):
        b, h = divmod(c, 2)
        p = np.asarray(prediction[b, h * _HALF : (h + 1) * _HALF], dtype=np.float32)
        g = np.asarray(ground_truth[b], dtype=np.float32)
        ap = np.empty((5, _HALF), dtype=np.float32)
        ap[0:3] = p.T
        ap[3] = (p * p).sum(axis=1, dtype=np.float32)
        ap[4] = 1.0
        ag = np.empty((5, _N), dtype=np.float32)
        ag[0:3] = (2.0 * g).T
        ag[3] = -1.0
        ag[4] = -(g * g).sum(axis=1, dtype=np.float32)
        in_maps.append({"ap": ap, "ag": ag})
    return in_maps


def _make_runner(nc, n_cores):
    """Build a cached jitted SPMD executor for `nc` (axon/PJRT path).

    Mirrors concourse.bass2jax.run_bass_via_pjrt but caches the jitted
    callable so repeat calls don't re-trace/re-compile.
    """
    import jax
    import numpy as _np
    from jax.sharding import Mesh, PartitionSpec
    from jax.experimental.shard_map import shard_map
    from concourse import mybir
    from concourse.bass2jax import (
        _bass_exec_p,
        install_neuronx_cc_hook,
        partition_id_tensor,
    )

    install_neuronx_cc_hook()

    partition_name = (
        nc.partition_id_tensor.name if nc.partition_id_tensor else None
    )
    in_names, out_names, out_avals, zero_shapes = [], [], [], []
    for alloc in nc.m.functions[0].allocations:
        if not isinstance(alloc, mybir.MemoryLocationSet):
            continue
        name = alloc.memorylocations[0].name
        if alloc.kind == "ExternalInput":
            if name == partition_name:
                continue
            in_names.append(name)
        elif alloc.kind == "ExternalOutput":
            shape = tuple(alloc.tensor_shape)
            dtype = mybir.dt.np(alloc.dtype)
            out_names.append(name)
            out_avals.append(jax.core.ShapedArray(shape, dtype))
            zero_shapes.append((shape, dtype))
    n_params = len(in_names)
    n_outs = len(out_names)
    all_names = in_names + out_names
    if partition_name is not None:
        all_names = all_names + [partition_name]
    donate = tuple(range(n_params, n_params + n_outs))

    def _body(*args):
        operands = list(args)
        if partition_name is not None:
            operands.append(partition_id_tensor())
        outs = _bass_exec_p.bind(
            *operands,
            out_avals=tuple(out_avals),
            in_names=tuple(all_names),
            out_names=tuple(out_names),
            lowering_input_output_aliases=(),
            sim_require_finite=True,
            sim_require_nnan=True,
            nc=nc,
        )
        return tuple(outs)

    devices = jax.devices()[:n_cores]
    mesh = Mesh(_np.asarray(devices), ("core",))
    sharded = jax.jit(
        shard_map(
            _body,
            mesh=mesh,
            in_specs=(PartitionSpec("core"),) * (n_params + n_outs),
            out_specs=(PartitionSpec("core"),) * n_outs,
            check_rep=False,
        ),
        donate_argnums=donate,
        keep_unused=True,
    )

    def run(in_maps):
        concat_in = [
            _np.concatenate([m[name] for m in in_maps], axis=0)
            for name in in_names
        ]
        concat_zeros = [
            _np.zeros((n_cores * s[0], *s[1:]), d) for (s, d) in zero_shapes
        ]
        out_arrs = sharded(*concat_in, *concat_zeros)
        return [
            {
                name: _np.asarray(out_arrs[i]).reshape(
                    n_cores, *out_avals[i].shape
                )[c]
                for i, name in enumerate(out_names)
            }
            for c in range(n_cores)
        ]

    return run


def _get_runner(nc, n_cores=_NCORES):
    key = id(nc)
    if key not in _RUNNERS:
        _RUNNERS[key] = _make_runner(nc, n_cores)
    return _RUNNERS[key]


def kernel(prediction, ground_truth):
    prediction = np.asarray(prediction, dtype=np.float32)
    ground_truth = np.asarray(ground_truth, dtype=np.float32)

    nc = _get_nc()
    in_maps = _prep_core_inputs(prediction, ground_truth)
    results = _get_runner(nc)(in_maps)

    out = np.zeros(_B, dtype=np.float32)
    for b in range(_B):
        dx = 0.0
        cms = []
        for h in range(2):
            r = results[2 * b + h]
            # rowparts[p, rb] = max over cols of -d2 for row rb*128+p
            rp = r["rowparts"].astype(np.float32)  # [128, RB]
            dx += np.maximum(-rp, 0.0).sum(dtype=np.float64)
            # colmax[p, m] = max over this core's rows (partition p) of -d2
            cms.append(r["colmax"].astype(np.float32).max(axis=0))  # [N]
        cm = np.maximum(cms[0], cms[1])
        dy = np.maximum(-cm, 0.0).sum(dtype=np.float64)
        out[b] = dx / _N + dy / _N
    return out
